# revision 29
# baseline (speedup 1.0000x reference)
"""Block-diagonal (per-frame) multi-head attention on 8 Trainium2 cores.

Problem: x[2,3200,512] -> QKV proj (H=8 heads, D=64) -> attention masked to
25-token frames (128 frames) -> out[2,3200,512].  N = 3200 = 128*25.

Sharding: 256 (batch, frame) groups; core c handles batch c//4, frames
(c%4)*32..+32  => 800 tokens/core, tiled as 8 x 100 tokens (4 frames).

v2 layout/schedule:
  - All stages in ONE pool scope so the Tile scheduler can overlap the
    QKV projections with attention tiles (no phase barrier).
  - Projections contract over the partition dim: qT/kT [feat, tok] =
    W.T @ xT with W-slices stationary; v [tok, feat] = xT.T @ Wv.
  - Per 100-token tile, scores live in TWO psum banks: stE [100, 4*100]
    holds the 4 even heads (PE rows 0-63), stO the odd heads (rows
    64-127) -- separate banks so the PE's row-group-concurrent matmuls
    never co-write a bank.  A rank-5 mask matmul (f16-safe +-30000)
    initializes each bank; exp is ONE activation per bank.
  - v has a ones-column per head so PV's last column yields the softmax
    denominator; per tile ONE reciprocal + ONE broadcast multiply
    produce the normalized output.
  - Outputs DMA on the gpsimd queue so they don't head-block inputs.
"""

import numpy as np

B, N, DIN = 2, 3200, 512
H, D = 8, 64
TL, JN = 128, 25
NCORES = 8
TOK = 800      # tokens per core
NT = 8         # token tiles per core
TT = 100       # tokens per tile (4 frames)
CH = 400       # proj column-chunk (2 chunks)
NEGB = 30000.0  # additive mask magnitude (f16-safe; |scores| <~ 10)

# matmul dtype per stage: 'f32' | 'f32r' | 'bf16' | 'f16'
CONFIG = {"proj": "f16", "qk": "f16", "pv": "f16"}
NWARM = 48     # PE-warmup filler matmuls during the input-DMA lead-in

_CACHE = {}
LAST_RESULT = None  # BassKernelResults of the most recent kernel() call


def _build(cfg):
    import concourse.bacc as bacc
    import concourse.tile as tile
    from concourse import mybir
    from concourse.bass import broadcast_tensor_aps

    f32 = mybir.dt.float32
    bf16 = mybir.dt.bfloat16
    f16 = mybir.dt.float16
    f32r = mybir.dt.float32r
    AF = mybir.ActivationFunctionType
    ALU = mybir.AluOpType

    def io_dt(kind):
        return {"f32": f32, "f32r": f32r, "bf16": bf16, "f16": f16}[kind]

    proj_dt = io_dt(cfg["proj"])
    qk_dt = io_dt(cfg["qk"])
    pv_dt = io_dt(cfg["pv"])
    mask_dt = f16 if cfg["qk"] == "f16" else bf16

    nc = bacc.Bacc("TRN2", target_bir_lowering=False, debug=False,
                   num_devices=NCORES)

    # packed layouts: k-slices side by side so every DMA row is >=2KB
    xt_d = nc.dram_tensor("xTp", [128, 4 * TOK], proj_dt,
                          kind="ExternalInput").ap()
    w_d = {}
    for nm in ("wq", "wk", "wv"):
        w_d[nm] = nc.dram_tensor(nm, [128, 4 * DIN], proj_dt,
                                 kind="ExternalInput").ap()
    bqc_d = nc.dram_tensor("bqc", [128, 4], f32, kind="ExternalInput").ap()
    bkc_d = nc.dram_tensor("bkc", [128, 4], f32, kind="ExternalInput").ap()
    bvr_d = nc.dram_tensor("bvr", [1, DIN], f32, kind="ExternalInput").ap()
    ma_d = nc.dram_tensor("mA", [5, TT], mask_dt, kind="ExternalInput").ap()
    mb4_d = nc.dram_tensor("mB4", [5, 4 * TT], mask_dt,
                           kind="ExternalInput").ap()
    out_d = nc.dram_tensor("out", [TOK, DIN], f16,
                          kind="ExternalOutput").ap()

    with tile.TileContext(nc) as tc:
        with (
            tc.tile_pool(name="pp", bufs=1) as pp,
            tc.tile_pool(name="sp", bufs=4) as sp,
            tc.tile_pool(name="ps", bufs=2, space="PSUM") as ps,
        ):
            # ---- persistent tiles (packed: k-slices side by side) ----
            wq_all = pp.tile([128, 4 * DIN], proj_dt, name="wq_all",
                             tag="wq_all")
            wk_all = pp.tile([128, 4 * DIN], proj_dt, name="wk_all",
                             tag="wk_all")
            wv_all = pp.tile([128, 4 * DIN], proj_dt, name="wv_all",
                             tag="wv_all")
            xt_all = pp.tile([128, 4 * TOK], proj_dt, name="xt_all",
                             tag="xt_all")
            wq = [wq_all[:, k * DIN:(k + 1) * DIN] for k in range(4)]
            wk = [wk_all[:, k * DIN:(k + 1) * DIN] for k in range(4)]
            wv = [wv_all[:, k * DIN:(k + 1) * DIN] for k in range(4)]
            xt = [xt_all[:, k * TOK:(k + 1) * TOK] for k in range(4)]
            bqc = pp.tile([128, 4], f32, name="bqc", tag="bqc")
            bkc = pp.tile([128, 4], f32, name="bkc", tag="bkc")
            bvb = pp.tile([128, DIN], f32, name="bvb", tag="bvb")
            ma = pp.tile([128, TT], mask_dt, name="ma", tag="ma")
            mb4 = pp.tile([128, 4 * TT], mask_dt, name="mb4", tag="mb4")

            qt = [pp.tile([128, TOK], qk_dt, name=f"qt{k}", tag=f"qt{k}")
                  for k in range(4)]
            kt_ = [pp.tile([128, TOK], qk_dt, name=f"kt{k}", tag=f"kt{k}")
                   for k in range(4)]
            # v with 65 columns per head: col h*65+64 is all-ones so the PV
            # matmul also produces the softmax denominator in its last column
            vt = [pp.tile([TT, H * (D + 1)], pv_dt, name=f"vt{t}",
                          tag=f"vt{t}") for t in range(NT)]
            ot = [pp.tile([TT, DIN], f16, name=f"ot{t}", tag=f"ot{t}")
                  for t in range(NT)]

            # ---- PE warm-up: junk matmuls keep the PE HAM-busy from t~0
            # so the clock is at 8/8 when real work arrives.  They write a
            # psum slot ('pv' tag) whose first real use is ~15us in, and
            # read a memset tile, so they gate nothing.
            junk = pp.tile([128, 256], qk_dt, name="junk", tag="junk")
            nc.gpsimd.memset(junk[:], 0.0)
            wacc = ps.tile([TT, 1024], f32, name="wacc", tag="pv", bufs=1)
            for i in range(NWARM):
                nc.tensor.matmul(wacc[:, 0:128], junk[:, 0:TT],
                                 junk[:, 0:128], start=True, stop=True,
                                 skip_group_check=True)

            # ---- input DMAs: two hw queues stream concurrently; each
            # queue is in priority order.  Weights on sync, x + small
            # tensors on scalar, so the critical (wq, xt) pair shares the
            # full HBM port instead of serializing on one queue.
            # weights split in halves so the completion semaphores unlock
            # the k=0,1 accumulation matmuls before the full tensor lands
            nc.sync.dma_start(out=wq_all[:, 0:2 * DIN],
                              in_=w_d["wq"][:, 0:2 * DIN])
            nc.sync.dma_start(out=wq_all[:, 2 * DIN:4 * DIN],
                              in_=w_d["wq"][:, 2 * DIN:4 * DIN])
            nc.sync.dma_start(out=wk_all[:, 0:2 * DIN],
                              in_=w_d["wk"][:, 0:2 * DIN])
            nc.sync.dma_start(out=wk_all[:, 2 * DIN:4 * DIN],
                              in_=w_d["wk"][:, 2 * DIN:4 * DIN])
            nc.sync.dma_start(out=wv_all, in_=w_d["wv"])
            # x split by column-chunk: each DMA covers cols [ch*CH, ch*CH+CH)
            # of all 4 k-slices (strided rows of 4x800B -- still big rows).
            # Triggers go on the gpsimd engine (idle early) so they don't
            # head-block the scalar (ACT) instruction stream.
            xa = xt_all.rearrange("p (k t) -> p k t", t=TOK)
            xd = xt_d.rearrange("p (k t) -> p k t", t=TOK)
            nc.gpsimd.dma_start(out=xa[:, :, 0:CH], in_=xd[:, :, 0:CH])
            nc.gpsimd.dma_start(out=xa[:, :, CH:TOK], in_=xd[:, :, CH:TOK])
            nc.gpsimd.dma_start(out=bqc, in_=bqc_d)
            nc.gpsimd.dma_start(out=bkc, in_=bkc_d)
            nc.gpsimd.dma_start(out=ma[0:5, :], in_=ma_d)
            nc.gpsimd.dma_start(out=ma[64:69, :], in_=ma_d)
            nc.gpsimd.dma_start(out=mb4[0:5, :], in_=mb4_d)
            nc.gpsimd.dma_start(out=mb4[64:69, :], in_=mb4_d)
            # broadcast the v-bias row to 100 partitions straight from DRAM
            # (stride-0 partition dim on the source AP)
            bv_src, _ = broadcast_tensor_aps(bvr_d, bvb[0:TT, :])
            nc.gpsimd.dma_start(out=bvb[0:TT, :], in_=bv_src)

            # ---- stage emitters ----
            def qk_group(w, bc, dst, ft, ch):
                fsl = slice(ft * 128, (ft + 1) * 128)
                csl = slice(ch * CH, (ch + 1) * CH)
                acc = ps.tile([128, CH], f32, name="acc", tag="acc", bufs=2)
                for k in range(4):
                    nc.tensor.matmul(acc[:], w[k][:, fsl], xt[k][:, csl],
                                     start=(k == 0), stop=(k == 3))
                # psum->sbuf copy + per-partition bias on DVE, keeping the
                # scalar engine free for the exp activations
                nc.vector.tensor_scalar_add(dst[ft][:, csl], acc[:],
                                            bc[:, ft:ft + 1])

            def v_tile(t):
                tsl = slice(t * TT, (t + 1) * TT)
                acc = ps.tile([TT, DIN], f32, name="vacc", tag="vacc", bufs=1)
                for k in range(4):
                    nc.tensor.matmul(acc[:], xt[k][:, tsl], wv[k][:],
                                     start=(k == 0), stop=(k == 3))
                vv = vt[t].rearrange("p (h c) -> p h c", c=D + 1)
                av = acc.rearrange("p (h c) -> p h c", c=D)
                bv = bvb[:TT, :].rearrange("p (h c) -> p h c", c=D)
                nc.vector.scalar_tensor_tensor(vv[:, :, :D], av, 0.0, bv,
                                               op0=ALU.add, op1=ALU.add)
                nc.vector.tensor_scalar_max(vv[:, :, :D], vv[:, :, :D], 0.0)
                nc.vector.memset(vv[:, :, D:D + 1], 1.0)

            def att_tile(t):
                tsl = slice(t * TT, (t + 1) * TT)
                # two banks: even heads (PE rows 0-63) / odd heads (64-127)
                stE = ps.tile([TT, 4 * TT], f32, name="stE", tag="st",
                              bufs=3)
                stO = ps.tile([TT, 4 * TT], f32, name="stO", tag="st",
                              bufs=3)
                nc.tensor.matmul(stE[:], ma[0:5, :], mb4[0:5, :],
                                 start=True, stop=False,
                                 skip_group_check=True)
                nc.tensor.matmul(stO[:], ma[64:69, :], mb4[64:69, :],
                                 start=True, stop=False,
                                 skip_group_check=True)
                for i in range(4):
                    c = slice(i * TT, (i + 1) * TT)
                    # head 2i: ft=i rows 0-63; head 2i+1: ft=i rows 64-127
                    nc.tensor.matmul(stE[:, c], kt_[i][0:64, tsl],
                                     qt[i][0:64, tsl],
                                     start=False, stop=(i == 3),
                                     skip_group_check=True)
                    nc.tensor.matmul(stO[:, c], kt_[i][64:128, tsl],
                                     qt[i][64:128, tsl],
                                     start=False, stop=(i == 3),
                                     skip_group_check=True)
                etE = sp.tile([TT, 4 * TT], pv_dt, name="etE", tag="et",
                              bufs=4)
                etO = sp.tile([TT, 4 * TT], pv_dt, name="etO", tag="et",
                              bufs=4)
                nc.scalar.activation(etE[:], stE[:], AF.Exp)
                nc.scalar.activation(etO[:], stO[:], AF.Exp)

                # PV: 2-bank psum [100, 2x512]; head h at bank h//4,
                # col (h%4)*65 (65 cols incl denominator)
                pv = ps.tile([TT, 1024], f32, name="pv", tag="pv", bufs=1)
                for h in range(H):
                    et = etE if h % 2 == 0 else etO
                    blk = h // 2
                    off = (h // 4) * 512 + (h % 4) * 65
                    nc.tensor.matmul(pv[:, off:off + 65],
                                     et[:, blk * TT:(blk + 1) * TT],
                                     vt[t][:, h * 65:(h + 1) * 65],
                                     start=True, stop=True,
                                     skip_group_check=True)
                pvb = pv.rearrange("p (b s) -> p b s", s=512)
                pvq = pvb[:, :, 0:4 * 65].rearrange("p b (q c) -> p b q c",
                                                    c=65)
                rc = sp.tile([TT, 8], f32, name="rc", tag="rc", bufs=4)
                rcv = rc.rearrange("p (b q c) -> p b q c", b=2, c=1)
                nc.vector.reciprocal(rcv, pvq[:, :, :, D:D + 1])
                ov = ot[t].rearrange("p (b q c) -> p b q c", b=2, c=D)
                i0, i1 = broadcast_tensor_aps(pvq[:, :, :, 0:D], rcv)
                nc.vector.tensor_tensor(ov, i0, i1, op=ALU.mult)
                nc.sync.dma_start(out=out_d[tsl, :], in_=ot[t][:])

            # ---- pipelined emission (matches DMA arrival order) ----
            for ch in range(2):
                for ft in range(4):
                    qk_group(wq, bqc, qt, ft, ch)
            for ch in range(2):
                for ft in range(4):
                    qk_group(wk, bkc, kt_, ft, ch)
            v_tile(0)
            v_tile(1)
            v_tile(2)
            att_tile(0)
            v_tile(3)
            att_tile(1)
            v_tile(4)
            att_tile(2)
            v_tile(5)
            att_tile(3)
            v_tile(6)
            att_tile(4)
            v_tile(7)
            att_tile(5)
            att_tile(6)
            att_tile(7)

    nc.compile()
    return nc


def _prep_inputs(x, Wq, bq, Wk, bk, Wv, bv, cfg):
    import ml_dtypes

    x = np.asarray(x, np.float32)
    Wq = np.asarray(Wq, np.float32)
    bq = np.asarray(bq, np.float32)
    Wk = np.asarray(Wk, np.float32)
    bk = np.asarray(bk, np.float32)
    Wv = np.asarray(Wv, np.float32)
    bv = np.asarray(bv, np.float32)

    scale = 1.0 / np.sqrt(np.float32(D))  # 1/8, exact
    wq_s = (Wq * scale).astype(np.float32)
    bq_s = (bq * scale).astype(np.float32)

    io_np = {"bf16": ml_dtypes.bfloat16,
             "f16": np.float16}.get(cfg["proj"], np.float32)
    mask_np = np.float16 if cfg["qk"] == "f16" else ml_dtypes.bfloat16
    xT = np.ascontiguousarray(x.transpose(0, 2, 1))  # [B, DIN, N]

    bqc = np.ascontiguousarray(bq_s.reshape(4, 128).T)
    bkc = np.ascontiguousarray(bk.reshape(4, 128).T)
    bvr = np.ascontiguousarray(bv[None, :])

    # rank-5 factors of the additive frame mask over one 100-token tile
    # (the kernel DMAs these 5 rows to partition bases 0 and 64)
    big = mask_np(NEGB)
    mA = np.zeros((5, TT), mask_np)
    mB = np.zeros((5, TT), mask_np)
    mA[0, :] = 1
    mB[0, :] = -big
    for f in range(4):
        mA[1 + f, f * JN:(f + 1) * JN] = 1
        mB[1 + f, f * JN:(f + 1) * JN] = big
    mB4 = np.ascontiguousarray(np.tile(mB, (1, 4)))

    def pack_w(w):
        # [512, 512] -> [128, 4*512]: k-slices side by side (2KB+ DMA rows)
        return np.ascontiguousarray(
            w.reshape(4, 128, DIN).transpose(1, 0, 2).reshape(128, 4 * DIN)
        ).astype(io_np)

    wq_p, wk_p, wv_p = pack_w(wq_s), pack_w(Wk), pack_w(Wv)

    in_maps = []
    for c in range(NCORES):
        b, fb = c // 4, c % 4
        xc = xT[b, :, fb * TOK:(fb + 1) * TOK]  # [512, 800]
        xt_p = np.ascontiguousarray(
            xc.reshape(4, 128, TOK).transpose(1, 0, 2).reshape(128, 4 * TOK)
        ).astype(io_np)
        in_maps.append({
            "xTp": xt_p,
            "wq": wq_p,
            "wk": wk_p,
            "wv": wv_p,
            "bqc": bqc, "bkc": bkc, "bvr": bvr,
            "mA": mA, "mB4": mB4,
        })
    return in_maps


def kernel(x, Wq, bq, Wk, bk, Wv, bv, att_heads=H, latent_dim=D,
           time_len=TL, joint_num=JN, **_):
    from concourse.bass_utils import run_bass_kernel_spmd

    cfg = tuple(sorted(CONFIG.items()))
    if cfg not in _CACHE:
        _CACHE[cfg] = _build(CONFIG)
    nc = _CACHE[cfg]

    in_maps = _prep_inputs(x, Wq, bq, Wk, bk, Wv, bv, CONFIG)
    res = run_bass_kernel_spmd(nc, in_maps, core_ids=list(range(NCORES)))
    global LAST_RESULT
    LAST_RESULT = res

    out = np.empty((B, N, DIN), np.float32)
    for c in range(NCORES):
        b, fb = c // 4, c % 4
        out[b, fb * TOK:(fb + 1) * TOK, :] = res.results[c]["out"]
    return out


# revision 30
# speedup vs baseline: 1.1017x; 1.1017x over previous
"""Block-diagonal (per-frame) multi-head attention on 8 Trainium2 cores.

Problem: x[2,3200,512] -> QKV proj (H=8 heads, D=64) -> attention masked to
25-token frames (128 frames) -> out[2,3200,512].  N = 3200 = 128*25.

Sharding: 256 (batch, frame) groups; core c handles batch c//4, frames
(c%4)*32..+32  => 800 tokens/core, tiled as 8 x 100 tokens (4 frames).

v2 layout/schedule:
  - All stages in ONE pool scope so the Tile scheduler can overlap the
    QKV projections with attention tiles (no phase barrier).
  - Projections contract over the partition dim: qT/kT [feat, tok] =
    W.T @ xT with W-slices stationary; v [tok, feat] = xT.T @ Wv.
  - Per 100-token tile, scores live in TWO psum banks: stE [100, 4*100]
    holds the 4 even heads (PE rows 0-63), stO the odd heads (rows
    64-127) -- separate banks so the PE's row-group-concurrent matmuls
    never co-write a bank.  A rank-5 mask matmul (f16-safe +-30000)
    initializes each bank; exp is ONE activation per bank.
  - v has a ones-column per head so PV's last column yields the softmax
    denominator; per tile ONE reciprocal + ONE broadcast multiply
    produce the normalized output.
  - Outputs DMA on the gpsimd queue so they don't head-block inputs.
"""

import numpy as np

B, N, DIN = 2, 3200, 512
H, D = 8, 64
TL, JN = 128, 25
NCORES = 8
TOK = 800      # tokens per core
NT = 8         # token tiles per core
TT = 100       # tokens per tile (4 frames)
CH = 400       # proj column-chunk (2 chunks)
NEGB = 30000.0  # additive mask magnitude (f16-safe; |scores| <~ 10)

# matmul dtype per stage: 'f32' | 'f32r' | 'bf16' | 'f16'
CONFIG = {"proj": "f16", "qk": "f16", "pv": "f16"}
NWARM = 48     # PE-warmup filler matmuls during the input-DMA lead-in

_CACHE = {}
LAST_RESULT = None  # BassKernelResults of the most recent kernel() call


def _build(cfg):
    import concourse.bacc as bacc
    import concourse.tile as tile
    from concourse import mybir
    from concourse.bass import broadcast_tensor_aps

    f32 = mybir.dt.float32
    bf16 = mybir.dt.bfloat16
    f16 = mybir.dt.float16
    f32r = mybir.dt.float32r
    AF = mybir.ActivationFunctionType
    ALU = mybir.AluOpType

    def io_dt(kind):
        return {"f32": f32, "f32r": f32r, "bf16": bf16, "f16": f16}[kind]

    proj_dt = io_dt(cfg["proj"])
    qk_dt = io_dt(cfg["qk"])
    pv_dt = io_dt(cfg["pv"])
    mask_dt = f16 if cfg["qk"] == "f16" else bf16

    nc = bacc.Bacc("TRN2", target_bir_lowering=False, debug=False,
                   num_devices=NCORES)

    # packed layouts: k-slices side by side so every DMA row is >=2KB
    xt_d = nc.dram_tensor("xTp", [128, 4 * TOK], proj_dt,
                          kind="ExternalInput").ap()
    w_d = {}
    for nm in ("wq", "wk", "wv"):
        w_d[nm] = nc.dram_tensor(nm, [128, 4 * DIN], proj_dt,
                                 kind="ExternalInput").ap()
    bqc_d = nc.dram_tensor("bqc", [128, 4], f32, kind="ExternalInput").ap()
    bkc_d = nc.dram_tensor("bkc", [128, 4], f32, kind="ExternalInput").ap()
    bvr_d = nc.dram_tensor("bvr", [1, DIN], f32, kind="ExternalInput").ap()
    ma_d = nc.dram_tensor("mA", [5, TT], mask_dt, kind="ExternalInput").ap()
    mb4_d = nc.dram_tensor("mB4", [5, 4 * TT], mask_dt,
                           kind="ExternalInput").ap()
    out_d = nc.dram_tensor("out", [TOK, DIN], f16,
                          kind="ExternalOutput").ap()

    with tile.TileContext(nc) as tc:
        with (
            tc.tile_pool(name="pp", bufs=1) as pp,
            tc.tile_pool(name="sp", bufs=4) as sp,
            tc.tile_pool(name="ps", bufs=2, space="PSUM") as ps,
        ):
            # ---- persistent tiles (packed: k-slices side by side) ----
            wq_all = pp.tile([128, 4 * DIN], proj_dt, name="wq_all",
                             tag="wq_all")
            wk_all = pp.tile([128, 4 * DIN], proj_dt, name="wk_all",
                             tag="wk_all")
            wv_all = pp.tile([128, 4 * DIN], proj_dt, name="wv_all",
                             tag="wv_all")
            xt_all = pp.tile([128, 4 * TOK], proj_dt, name="xt_all",
                             tag="xt_all")
            wq = [wq_all[:, k * DIN:(k + 1) * DIN] for k in range(4)]
            wk = [wk_all[:, k * DIN:(k + 1) * DIN] for k in range(4)]
            wv = [wv_all[:, k * DIN:(k + 1) * DIN] for k in range(4)]
            xt = [xt_all[:, k * TOK:(k + 1) * TOK] for k in range(4)]
            bqc = pp.tile([128, 4], f32, name="bqc", tag="bqc")
            bkc = pp.tile([128, 4], f32, name="bkc", tag="bkc")
            bvb = pp.tile([128, DIN], f32, name="bvb", tag="bvb")
            ma = pp.tile([128, TT], mask_dt, name="ma", tag="ma")
            mb4 = pp.tile([128, 4 * TT], mask_dt, name="mb4", tag="mb4")

            qt = [pp.tile([128, TOK], qk_dt, name=f"qt{k}", tag=f"qt{k}")
                  for k in range(4)]
            kt_ = [pp.tile([128, TOK], qk_dt, name=f"kt{k}", tag=f"kt{k}")
                   for k in range(4)]
            # v with 65 columns per head: col h*65+64 is all-ones so the PV
            # matmul also produces the softmax denominator in its last column
            vt = [pp.tile([TT, H * (D + 1)], pv_dt, name=f"vt{t}",
                          tag=f"vt{t}") for t in range(NT)]
            ot = [pp.tile([TT, DIN], f16, name=f"ot{t}", tag=f"ot{t}")
                  for t in range(NT)]

            # ---- PE warm-up: junk matmuls keep the PE HAM-busy from t~0
            # so the clock is at 8/8 when real work arrives.  They write a
            # psum slot ('pv' tag) whose first real use is ~15us in, and
            # read a memset tile, so they gate nothing.
            junk = pp.tile([128, 256], qk_dt, name="junk", tag="junk")
            nc.gpsimd.memset(junk[:], 0.0)
            wacc = ps.tile([TT, 1024], f32, name="wacc", tag="pv", bufs=1)
            for i in range(NWARM):
                nc.tensor.matmul(wacc[:, 0:128], junk[:, 0:TT],
                                 junk[:, 0:128], start=True, stop=True,
                                 skip_group_check=True)

            # ---- input DMAs: ONE hw queue (sync) in strict priority
            # order matching the PE emission order, so arrival times are
            # deterministic and each projection phase unblocks in turn.
            # Small constants ride the gpsimd queue (idle early).
            xa = xt_all.rearrange("p (k t) -> p k t", t=TOK)
            xd = xt_d.rearrange("p (k t) -> p k t", t=TOK)
            nc.sync.dma_start(out=wq_all, in_=w_d["wq"])
            nc.sync.dma_start(out=xa[:, :, 0:CH], in_=xd[:, :, 0:CH])
            nc.sync.dma_start(out=wk_all, in_=w_d["wk"])
            nc.sync.dma_start(out=xa[:, :, CH:TOK], in_=xd[:, :, CH:TOK])
            nc.sync.dma_start(out=wv_all, in_=w_d["wv"])
            nc.gpsimd.dma_start(out=bqc, in_=bqc_d)
            nc.gpsimd.dma_start(out=bkc, in_=bkc_d)
            nc.gpsimd.dma_start(out=ma[0:5, :], in_=ma_d)
            nc.gpsimd.dma_start(out=ma[64:69, :], in_=ma_d)
            nc.gpsimd.dma_start(out=mb4[0:5, :], in_=mb4_d)
            nc.gpsimd.dma_start(out=mb4[64:69, :], in_=mb4_d)
            # broadcast the v-bias row to 100 partitions straight from DRAM
            # (stride-0 partition dim on the source AP)
            bv_src, _ = broadcast_tensor_aps(bvr_d, bvb[0:TT, :])
            nc.gpsimd.dma_start(out=bvb[0:TT, :], in_=bv_src)

            def pad(n):
                # HAM-keepalive padding between phases: keeps the PE from
                # idling into a re-throttle if the next DMA is late
                for _ in range(n):
                    nc.tensor.matmul(wacc[:, 0:128], junk[:, 0:TT],
                                     junk[:, 0:128], start=True, stop=True,
                                     skip_group_check=True)

            # ---- stage emitters ----
            def qk_group(w, bc, dst, ft, ch):
                fsl = slice(ft * 128, (ft + 1) * 128)
                csl = slice(ch * CH, (ch + 1) * CH)
                acc = ps.tile([128, CH], f32, name="acc", tag="acc", bufs=2)
                for k in range(4):
                    nc.tensor.matmul(acc[:], w[k][:, fsl], xt[k][:, csl],
                                     start=(k == 0), stop=(k == 3))
                # psum->sbuf copy + per-partition bias on DVE, keeping the
                # scalar engine free for the exp activations
                nc.vector.tensor_scalar_add(dst[ft][:, csl], acc[:],
                                            bc[:, ft:ft + 1])

            def v_tile(t):
                tsl = slice(t * TT, (t + 1) * TT)
                acc = ps.tile([TT, DIN], f32, name="vacc", tag="vacc", bufs=1)
                for k in range(4):
                    nc.tensor.matmul(acc[:], xt[k][:, tsl], wv[k][:],
                                     start=(k == 0), stop=(k == 3))
                vv = vt[t].rearrange("p (h c) -> p h c", c=D + 1)
                av = acc.rearrange("p (h c) -> p h c", c=D)
                bv = bvb[:TT, :].rearrange("p (h c) -> p h c", c=D)
                nc.vector.scalar_tensor_tensor(vv[:, :, :D], av, 0.0, bv,
                                               op0=ALU.add, op1=ALU.add)
                nc.vector.tensor_scalar_max(vv[:, :, :D], vv[:, :, :D], 0.0)
                nc.vector.memset(vv[:, :, D:D + 1], 1.0)

            def att_tile(t):
                tsl = slice(t * TT, (t + 1) * TT)
                # two banks: even heads (PE rows 0-63) / odd heads (64-127)
                stE = ps.tile([TT, 4 * TT], f32, name="stE", tag="st",
                              bufs=3)
                stO = ps.tile([TT, 4 * TT], f32, name="stO", tag="st",
                              bufs=3)
                nc.tensor.matmul(stE[:], ma[0:5, :], mb4[0:5, :],
                                 start=True, stop=False,
                                 skip_group_check=True)
                nc.tensor.matmul(stO[:], ma[64:69, :], mb4[64:69, :],
                                 start=True, stop=False,
                                 skip_group_check=True)
                for i in range(4):
                    c = slice(i * TT, (i + 1) * TT)
                    # head 2i: ft=i rows 0-63; head 2i+1: ft=i rows 64-127
                    nc.tensor.matmul(stE[:, c], kt_[i][0:64, tsl],
                                     qt[i][0:64, tsl],
                                     start=False, stop=(i == 3),
                                     skip_group_check=True)
                    nc.tensor.matmul(stO[:, c], kt_[i][64:128, tsl],
                                     qt[i][64:128, tsl],
                                     start=False, stop=(i == 3),
                                     skip_group_check=True)
                etE = sp.tile([TT, 4 * TT], pv_dt, name="etE", tag="et",
                              bufs=4)
                etO = sp.tile([TT, 4 * TT], pv_dt, name="etO", tag="et",
                              bufs=4)
                nc.scalar.activation(etE[:], stE[:], AF.Exp)
                nc.scalar.activation(etO[:], stO[:], AF.Exp)

                # PV: 2-bank psum [100, 2x512]; head h at bank h//4,
                # col (h%4)*65 (65 cols incl denominator)
                pv = ps.tile([TT, 1024], f32, name="pv", tag="pv", bufs=1)
                for h in range(H):
                    et = etE if h % 2 == 0 else etO
                    blk = h // 2
                    off = (h // 4) * 512 + (h % 4) * 65
                    nc.tensor.matmul(pv[:, off:off + 65],
                                     et[:, blk * TT:(blk + 1) * TT],
                                     vt[t][:, h * 65:(h + 1) * 65],
                                     start=True, stop=True,
                                     skip_group_check=True)
                pvb = pv.rearrange("p (b s) -> p b s", s=512)
                pvq = pvb[:, :, 0:4 * 65].rearrange("p b (q c) -> p b q c",
                                                    c=65)
                rc = sp.tile([TT, 8], f32, name="rc", tag="rc", bufs=4)
                rcv = rc.rearrange("p (b q c) -> p b q c", b=2, c=1)
                nc.vector.reciprocal(rcv, pvq[:, :, :, D:D + 1])
                ov = ot[t].rearrange("p (b q c) -> p b q c", b=2, c=D)
                i0, i1 = broadcast_tensor_aps(pvq[:, :, :, 0:D], rcv)
                nc.vector.tensor_tensor(ov, i0, i1, op=ALU.mult)
                nc.sync.dma_start(out=out_d[tsl, :], in_=ot[t][:])

            # ---- pipelined emission (matches DMA arrival order) ----
            for ft in range(4):
                qk_group(wq, bqc, qt, ft, 0)
            pad(8)
            for ft in range(4):
                qk_group(wk, bkc, kt_, ft, 0)
            pad(8)
            for ft in range(4):
                qk_group(wq, bqc, qt, ft, 1)
            for ft in range(4):
                qk_group(wk, bkc, kt_, ft, 1)
            pad(8)
            v_tile(0)
            v_tile(1)
            v_tile(2)
            att_tile(0)
            v_tile(3)
            att_tile(1)
            v_tile(4)
            att_tile(2)
            v_tile(5)
            att_tile(3)
            v_tile(6)
            att_tile(4)
            v_tile(7)
            att_tile(5)
            att_tile(6)
            att_tile(7)

    nc.compile()
    return nc


def _prep_inputs(x, Wq, bq, Wk, bk, Wv, bv, cfg):
    import ml_dtypes

    x = np.asarray(x, np.float32)
    Wq = np.asarray(Wq, np.float32)
    bq = np.asarray(bq, np.float32)
    Wk = np.asarray(Wk, np.float32)
    bk = np.asarray(bk, np.float32)
    Wv = np.asarray(Wv, np.float32)
    bv = np.asarray(bv, np.float32)

    scale = 1.0 / np.sqrt(np.float32(D))  # 1/8, exact
    wq_s = (Wq * scale).astype(np.float32)
    bq_s = (bq * scale).astype(np.float32)

    io_np = {"bf16": ml_dtypes.bfloat16,
             "f16": np.float16}.get(cfg["proj"], np.float32)
    mask_np = np.float16 if cfg["qk"] == "f16" else ml_dtypes.bfloat16
    xT = np.ascontiguousarray(x.transpose(0, 2, 1))  # [B, DIN, N]

    bqc = np.ascontiguousarray(bq_s.reshape(4, 128).T)
    bkc = np.ascontiguousarray(bk.reshape(4, 128).T)
    bvr = np.ascontiguousarray(bv[None, :])

    # rank-5 factors of the additive frame mask over one 100-token tile
    # (the kernel DMAs these 5 rows to partition bases 0 and 64)
    big = mask_np(NEGB)
    mA = np.zeros((5, TT), mask_np)
    mB = np.zeros((5, TT), mask_np)
    mA[0, :] = 1
    mB[0, :] = -big
    for f in range(4):
        mA[1 + f, f * JN:(f + 1) * JN] = 1
        mB[1 + f, f * JN:(f + 1) * JN] = big
    mB4 = np.ascontiguousarray(np.tile(mB, (1, 4)))

    def pack_w(w):
        # [512, 512] -> [128, 4*512]: k-slices side by side (2KB+ DMA rows)
        return np.ascontiguousarray(
            w.reshape(4, 128, DIN).transpose(1, 0, 2).reshape(128, 4 * DIN)
        ).astype(io_np)

    wq_p, wk_p, wv_p = pack_w(wq_s), pack_w(Wk), pack_w(Wv)

    in_maps = []
    for c in range(NCORES):
        b, fb = c // 4, c % 4
        xc = xT[b, :, fb * TOK:(fb + 1) * TOK]  # [512, 800]
        xt_p = np.ascontiguousarray(
            xc.reshape(4, 128, TOK).transpose(1, 0, 2).reshape(128, 4 * TOK)
        ).astype(io_np)
        in_maps.append({
            "xTp": xt_p,
            "wq": wq_p,
            "wk": wk_p,
            "wv": wv_p,
            "bqc": bqc, "bkc": bkc, "bvr": bvr,
            "mA": mA, "mB4": mB4,
        })
    return in_maps


def kernel(x, Wq, bq, Wk, bk, Wv, bv, att_heads=H, latent_dim=D,
           time_len=TL, joint_num=JN, **_):
    from concourse.bass_utils import run_bass_kernel_spmd

    cfg = tuple(sorted(CONFIG.items()))
    if cfg not in _CACHE:
        _CACHE[cfg] = _build(CONFIG)
    nc = _CACHE[cfg]

    in_maps = _prep_inputs(x, Wq, bq, Wk, bk, Wv, bv, CONFIG)
    res = run_bass_kernel_spmd(nc, in_maps, core_ids=list(range(NCORES)))
    global LAST_RESULT
    LAST_RESULT = res

    out = np.empty((B, N, DIN), np.float32)
    for c in range(NCORES):
        b, fb = c // 4, c % 4
        out[b, fb * TOK:(fb + 1) * TOK, :] = res.results[c]["out"]
    return out


# revision 31
# speedup vs baseline: 1.1123x; 1.0096x over previous
"""Block-diagonal (per-frame) multi-head attention on 8 Trainium2 cores.

Problem: x[2,3200,512] -> QKV proj (H=8 heads, D=64) -> attention masked to
25-token frames (128 frames) -> out[2,3200,512].  N = 3200 = 128*25.

Sharding: 256 (batch, frame) groups; core c handles batch c//4, frames
(c%4)*32..+32  => 800 tokens/core, tiled as 8 x 100 tokens (4 frames).

v2 layout/schedule:
  - All stages in ONE pool scope so the Tile scheduler can overlap the
    QKV projections with attention tiles (no phase barrier).
  - Projections contract over the partition dim: qT/kT [feat, tok] =
    W.T @ xT with W-slices stationary; v [tok, feat] = xT.T @ Wv.
  - Per 100-token tile, scores live in TWO psum banks: stE [100, 4*100]
    holds the 4 even heads (PE rows 0-63), stO the odd heads (rows
    64-127) -- separate banks so the PE's row-group-concurrent matmuls
    never co-write a bank.  A rank-5 mask matmul (f16-safe +-30000)
    initializes each bank; exp is ONE activation per bank.
  - v has a ones-column per head so PV's last column yields the softmax
    denominator; per tile ONE reciprocal + ONE broadcast multiply
    produce the normalized output.
  - Outputs DMA on the gpsimd queue so they don't head-block inputs.
"""

import numpy as np

B, N, DIN = 2, 3200, 512
H, D = 8, 64
TL, JN = 128, 25
NCORES = 8
TOK = 800      # tokens per core
NT = 8         # token tiles per core
TT = 100       # tokens per tile (4 frames)
CH = 400       # proj column-chunk (2 chunks)
NEGB = 30000.0  # additive mask magnitude (f16-safe; |scores| <~ 10)

# matmul dtype per stage: 'f32' | 'f32r' | 'bf16' | 'f16'
CONFIG = {"proj": "f16", "qk": "f16", "pv": "f16"}
NWARM = 48     # PE-warmup filler matmuls during the input-DMA lead-in

_CACHE = {}
LAST_RESULT = None  # BassKernelResults of the most recent kernel() call


def _build(cfg):
    import concourse.bacc as bacc
    import concourse.tile as tile
    from concourse import mybir
    from concourse.bass import broadcast_tensor_aps

    f32 = mybir.dt.float32
    bf16 = mybir.dt.bfloat16
    f16 = mybir.dt.float16
    f32r = mybir.dt.float32r
    AF = mybir.ActivationFunctionType
    ALU = mybir.AluOpType

    def io_dt(kind):
        return {"f32": f32, "f32r": f32r, "bf16": bf16, "f16": f16}[kind]

    proj_dt = io_dt(cfg["proj"])
    qk_dt = io_dt(cfg["qk"])
    pv_dt = io_dt(cfg["pv"])
    mask_dt = f16 if cfg["qk"] == "f16" else bf16

    nc = bacc.Bacc("TRN2", target_bir_lowering=False, debug=False,
                   num_devices=NCORES)

    # packed layouts: k-slices side by side so every DMA row is >=2KB
    xt_d = nc.dram_tensor("xTp", [128, 4 * TOK], proj_dt,
                          kind="ExternalInput").ap()
    w_d = {}
    for nm in ("wq", "wk", "wv"):
        w_d[nm] = nc.dram_tensor(nm, [128, 4 * DIN], proj_dt,
                                 kind="ExternalInput").ap()
    bqc_d = nc.dram_tensor("bqc", [128, 4], f32, kind="ExternalInput").ap()
    bkc_d = nc.dram_tensor("bkc", [128, 4], f32, kind="ExternalInput").ap()
    bvr_d = nc.dram_tensor("bvr", [1, DIN], f32, kind="ExternalInput").ap()
    ma_d = nc.dram_tensor("mA", [5, TT], mask_dt, kind="ExternalInput").ap()
    mb4_d = nc.dram_tensor("mB4", [5, 4 * TT], mask_dt,
                           kind="ExternalInput").ap()
    out_d = nc.dram_tensor("out", [TOK, DIN], f16,
                          kind="ExternalOutput").ap()

    with tile.TileContext(nc) as tc:
        with (
            tc.tile_pool(name="pp", bufs=1) as pp,
            tc.tile_pool(name="sp", bufs=4) as sp,
            tc.tile_pool(name="ps", bufs=2, space="PSUM") as ps,
        ):
            # ---- persistent tiles (packed: k-slices side by side) ----
            wq_all = pp.tile([128, 4 * DIN], proj_dt, name="wq_all",
                             tag="wq_all")
            wk_all = pp.tile([128, 4 * DIN], proj_dt, name="wk_all",
                             tag="wk_all")
            wv_all = pp.tile([128, 4 * DIN], proj_dt, name="wv_all",
                             tag="wv_all")
            xt_all = pp.tile([128, 4 * TOK], proj_dt, name="xt_all",
                             tag="xt_all")
            wq = [wq_all[:, k * DIN:(k + 1) * DIN] for k in range(4)]
            wk = [wk_all[:, k * DIN:(k + 1) * DIN] for k in range(4)]
            wv = [wv_all[:, k * DIN:(k + 1) * DIN] for k in range(4)]
            xt = [xt_all[:, k * TOK:(k + 1) * TOK] for k in range(4)]
            bqc = pp.tile([128, 4], f32, name="bqc", tag="bqc")
            bkc = pp.tile([128, 4], f32, name="bkc", tag="bkc")
            bvb = pp.tile([128, DIN], f32, name="bvb", tag="bvb")
            ma = pp.tile([128, TT], mask_dt, name="ma", tag="ma")
            mb4 = pp.tile([128, 4 * TT], mask_dt, name="mb4", tag="mb4")

            qt = [pp.tile([128, TOK], qk_dt, name=f"qt{k}", tag=f"qt{k}")
                  for k in range(4)]
            kt_ = [pp.tile([128, TOK], qk_dt, name=f"kt{k}", tag=f"kt{k}")
                   for k in range(4)]
            # v with 65 columns per head: col h*65+64 is all-ones so the PV
            # matmul also produces the softmax denominator in its last column
            vt = [pp.tile([TT, H * (D + 1)], pv_dt, name=f"vt{t}",
                          tag=f"vt{t}") for t in range(NT)]
            ot = [pp.tile([TT, DIN], f16, name=f"ot{t}", tag=f"ot{t}")
                  for t in range(NT)]

            # ---- PE warm-up: junk matmuls keep the PE HAM-busy from t~0
            # so the clock is at 8/8 when real work arrives.  They write a
            # psum slot ('pv' tag) whose first real use is ~15us in, and
            # read a memset tile, so they gate nothing.
            junk = pp.tile([128, 256], qk_dt, name="junk", tag="junk")
            nc.gpsimd.memset(junk[:], 0.0)
            wacc = ps.tile([TT, 1024], f32, name="wacc", tag="pv", bufs=1)
            for i in range(NWARM):
                nc.tensor.matmul(wacc[:, 0:128], junk[:, 0:TT],
                                 junk[:, 0:128], start=True, stop=True,
                                 skip_group_check=True)

            # ---- input DMAs: ONE hw queue (sync) in strict priority
            # order matching the PE emission order, so arrival times are
            # deterministic and each projection phase unblocks in turn.
            # Small constants ride the gpsimd queue (idle early).
            xa = xt_all.rearrange("p (k t) -> p k t", t=TOK)
            xd = xt_d.rearrange("p (k t) -> p k t", t=TOK)
            nc.sync.dma_start(out=wq_all, in_=w_d["wq"])
            nc.sync.dma_start(out=xa[:, :, 0:CH], in_=xd[:, :, 0:CH])
            nc.sync.dma_start(out=wk_all, in_=w_d["wk"])
            nc.sync.dma_start(out=xa[:, :, CH:TOK], in_=xd[:, :, CH:TOK])
            nc.sync.dma_start(out=wv_all, in_=w_d["wv"])
            nc.gpsimd.dma_start(out=bqc, in_=bqc_d)
            nc.gpsimd.dma_start(out=bkc, in_=bkc_d)
            nc.gpsimd.dma_start(out=ma[0:5, :], in_=ma_d)
            nc.gpsimd.dma_start(out=ma[64:69, :], in_=ma_d)
            nc.gpsimd.dma_start(out=mb4[0:5, :], in_=mb4_d)
            nc.gpsimd.dma_start(out=mb4[64:69, :], in_=mb4_d)
            # broadcast the v-bias row to 100 partitions straight from DRAM
            # (stride-0 partition dim on the source AP)
            bv_src, _ = broadcast_tensor_aps(bvr_d, bvb[0:TT, :])
            nc.gpsimd.dma_start(out=bvb[0:TT, :], in_=bv_src)

            def pad(n):
                # HAM-keepalive padding between phases: keeps the PE from
                # idling into a re-throttle if the next DMA is late
                for _ in range(n):
                    nc.tensor.matmul(wacc[:, 0:128], junk[:, 0:TT],
                                     junk[:, 0:128], start=True, stop=True,
                                     skip_group_check=True)

            # ---- stage emitters ----
            def qk_group(w, bc, dst, ft, ch):
                fsl = slice(ft * 128, (ft + 1) * 128)
                csl = slice(ch * CH, (ch + 1) * CH)
                acc = ps.tile([128, CH], f32, name="acc", tag="acc", bufs=2)
                for k in range(4):
                    nc.tensor.matmul(acc[:], w[k][:, fsl], xt[k][:, csl],
                                     start=(k == 0), stop=(k == 3))
                # psum->sbuf copy + per-partition bias on DVE, keeping the
                # scalar engine free for the exp activations
                nc.vector.tensor_scalar_add(dst[ft][:, csl], acc[:],
                                            bc[:, ft:ft + 1])

            def v_tile(t):
                tsl = slice(t * TT, (t + 1) * TT)
                acc = ps.tile([TT, DIN], f32, name="vacc", tag="vacc", bufs=1)
                for k in range(4):
                    nc.tensor.matmul(acc[:], xt[k][:, tsl], wv[k][:],
                                     start=(k == 0), stop=(k == 3))
                vv = vt[t].rearrange("p (h c) -> p h c", c=D + 1)
                av = acc.rearrange("p (h c) -> p h c", c=D)
                bv = bvb[:TT, :].rearrange("p (h c) -> p h c", c=D)
                nc.vector.scalar_tensor_tensor(vv[:, :, :D], av, 0.0, bv,
                                               op0=ALU.add, op1=ALU.add)
                nc.vector.tensor_scalar_max(vv[:, :, :D], vv[:, :, :D], 0.0)
                nc.vector.memset(vv[:, :, D:D + 1], 1.0)

            def att_tile(t, tagE="st", tagO="st"):
                tsl = slice(t * TT, (t + 1) * TT)
                # two banks: even heads (PE rows 0-63) / odd heads (64-127).
                # Late tiles borrow the proj accumulator banks (idle by
                # then; slot sizes are compatible) for deeper pipelining.
                stE = ps.tile([TT, 4 * TT], f32, name="stE", tag=tagE,
                              bufs={"st": 3, "acc": 2, "vacc": 1}[tagE])
                stO = ps.tile([TT, 4 * TT], f32, name="stO", tag=tagO,
                              bufs={"st": 3, "acc": 2, "vacc": 1}[tagO])
                nc.tensor.matmul(stE[:], ma[0:5, :], mb4[0:5, :],
                                 start=True, stop=False,
                                 skip_group_check=True)
                nc.tensor.matmul(stO[:], ma[64:69, :], mb4[64:69, :],
                                 start=True, stop=False,
                                 skip_group_check=True)
                for i in range(4):
                    c = slice(i * TT, (i + 1) * TT)
                    # head 2i: ft=i rows 0-63; head 2i+1: ft=i rows 64-127
                    nc.tensor.matmul(stE[:, c], kt_[i][0:64, tsl],
                                     qt[i][0:64, tsl],
                                     start=False, stop=(i == 3),
                                     skip_group_check=True)
                    nc.tensor.matmul(stO[:, c], kt_[i][64:128, tsl],
                                     qt[i][64:128, tsl],
                                     start=False, stop=(i == 3),
                                     skip_group_check=True)
                etE = sp.tile([TT, 4 * TT], pv_dt, name="etE", tag="et",
                              bufs=4)
                etO = sp.tile([TT, 4 * TT], pv_dt, name="etO", tag="et",
                              bufs=4)
                nc.scalar.activation(etE[:], stE[:], AF.Exp)
                nc.scalar.activation(etO[:], stO[:], AF.Exp)

                # PV: 2-bank psum [100, 2x512]; head h at bank h//4,
                # col (h%4)*65 (65 cols incl denominator)
                pv = ps.tile([TT, 1024], f32, name="pv", tag="pv", bufs=1)
                for h in range(H):
                    et = etE if h % 2 == 0 else etO
                    blk = h // 2
                    off = (h // 4) * 512 + (h % 4) * 65
                    nc.tensor.matmul(pv[:, off:off + 65],
                                     et[:, blk * TT:(blk + 1) * TT],
                                     vt[t][:, h * 65:(h + 1) * 65],
                                     start=True, stop=True,
                                     skip_group_check=True)
                pvb = pv.rearrange("p (b s) -> p b s", s=512)
                pvq = pvb[:, :, 0:4 * 65].rearrange("p b (q c) -> p b q c",
                                                    c=65)
                rc = sp.tile([TT, 8], f32, name="rc", tag="rc", bufs=4)
                rcv = rc.rearrange("p (b q c) -> p b q c", b=2, c=1)
                nc.vector.reciprocal(rcv, pvq[:, :, :, D:D + 1])
                ov = ot[t].rearrange("p (b q c) -> p b q c", b=2, c=D)
                i0, i1 = broadcast_tensor_aps(pvq[:, :, :, 0:D], rcv)
                nc.vector.tensor_tensor(ov, i0, i1, op=ALU.mult)
                nc.sync.dma_start(out=out_d[tsl, :], in_=ot[t][:])

            # ---- pipelined emission (matches DMA arrival order) ----
            for ft in range(4):
                qk_group(wq, bqc, qt, ft, 0)
            pad(8)
            for ft in range(4):
                qk_group(wk, bkc, kt_, ft, 0)
            pad(8)
            for ft in range(4):
                qk_group(wq, bqc, qt, ft, 1)
            for ft in range(4):
                qk_group(wk, bkc, kt_, ft, 1)
            pad(8)
            v_tile(0)
            v_tile(1)
            v_tile(2)
            att_tile(0)
            v_tile(3)
            att_tile(1)
            v_tile(4)
            att_tile(2)
            v_tile(5)
            att_tile(3)
            v_tile(6)
            att_tile(4)
            v_tile(7)
            att_tile(5, "acc", "acc")
            att_tile(6, "st", "st")
            att_tile(7, "vacc", "st")

    nc.compile()
    return nc


def _prep_inputs(x, Wq, bq, Wk, bk, Wv, bv, cfg):
    import ml_dtypes

    x = np.asarray(x, np.float32)
    Wq = np.asarray(Wq, np.float32)
    bq = np.asarray(bq, np.float32)
    Wk = np.asarray(Wk, np.float32)
    bk = np.asarray(bk, np.float32)
    Wv = np.asarray(Wv, np.float32)
    bv = np.asarray(bv, np.float32)

    scale = 1.0 / np.sqrt(np.float32(D))  # 1/8, exact
    wq_s = (Wq * scale).astype(np.float32)
    bq_s = (bq * scale).astype(np.float32)

    io_np = {"bf16": ml_dtypes.bfloat16,
             "f16": np.float16}.get(cfg["proj"], np.float32)
    mask_np = np.float16 if cfg["qk"] == "f16" else ml_dtypes.bfloat16
    xT = np.ascontiguousarray(x.transpose(0, 2, 1))  # [B, DIN, N]

    bqc = np.ascontiguousarray(bq_s.reshape(4, 128).T)
    bkc = np.ascontiguousarray(bk.reshape(4, 128).T)
    bvr = np.ascontiguousarray(bv[None, :])

    # rank-5 factors of the additive frame mask over one 100-token tile
    # (the kernel DMAs these 5 rows to partition bases 0 and 64)
    big = mask_np(NEGB)
    mA = np.zeros((5, TT), mask_np)
    mB = np.zeros((5, TT), mask_np)
    mA[0, :] = 1
    mB[0, :] = -big
    for f in range(4):
        mA[1 + f, f * JN:(f + 1) * JN] = 1
        mB[1 + f, f * JN:(f + 1) * JN] = big
    mB4 = np.ascontiguousarray(np.tile(mB, (1, 4)))

    def pack_w(w):
        # [512, 512] -> [128, 4*512]: k-slices side by side (2KB+ DMA rows)
        return np.ascontiguousarray(
            w.reshape(4, 128, DIN).transpose(1, 0, 2).reshape(128, 4 * DIN)
        ).astype(io_np)

    wq_p, wk_p, wv_p = pack_w(wq_s), pack_w(Wk), pack_w(Wv)

    in_maps = []
    for c in range(NCORES):
        b, fb = c // 4, c % 4
        xc = xT[b, :, fb * TOK:(fb + 1) * TOK]  # [512, 800]
        xt_p = np.ascontiguousarray(
            xc.reshape(4, 128, TOK).transpose(1, 0, 2).reshape(128, 4 * TOK)
        ).astype(io_np)
        in_maps.append({
            "xTp": xt_p,
            "wq": wq_p,
            "wk": wk_p,
            "wv": wv_p,
            "bqc": bqc, "bkc": bkc, "bvr": bvr,
            "mA": mA, "mB4": mB4,
        })
    return in_maps


def kernel(x, Wq, bq, Wk, bk, Wv, bv, att_heads=H, latent_dim=D,
           time_len=TL, joint_num=JN, **_):
    from concourse.bass_utils import run_bass_kernel_spmd

    cfg = tuple(sorted(CONFIG.items()))
    if cfg not in _CACHE:
        _CACHE[cfg] = _build(CONFIG)
    nc = _CACHE[cfg]

    in_maps = _prep_inputs(x, Wq, bq, Wk, bk, Wv, bv, CONFIG)
    res = run_bass_kernel_spmd(nc, in_maps, core_ids=list(range(NCORES)))
    global LAST_RESULT
    LAST_RESULT = res

    out = np.empty((B, N, DIN), np.float32)
    for c in range(NCORES):
        b, fb = c // 4, c % 4
        out[b, fb * TOK:(fb + 1) * TOK, :] = res.results[c]["out"]
    return out
